# revision 1
# baseline (speedup 1.0000x reference)
"""MultiHeadSemGConv Trainium2 kernel.

Computes, for x:[B,N,CIN], W:[H,2,CIN,HC], e:[H,N*K], bias:[H,HC],
rows/cols:[N*K] (int32 edge list):

    h = einsum('bnc,hscd->shbnd', x, W)             # two projections per head
    A = softmax(scatter(e at (rows,cols), NEG))     # [H,N,N]
    out[h,b] = diag(A)*h0 + (A - diag)@h1 + bias    # -> [B,N,H*HC]

Strategy: pure data-parallel over batch across 8 NeuronCores.  The tiny
[H,98,98] adjacency softmax is precomputed on host; the heavy lifting
(x projection + graph mixing over 100MB of activations) runs on device:

  per core (128 samples):
    - DMA x in flat 128-partition tiles, casting f32->fp16 in the DMA
    - PE transpose (matmul with identity) -> xT chunk tiles
      [c(2x128), 16*98+30 cols] fp16 in SBUF (30-col overlap keeps every
      per-sample phase-1 stationary at m=128)
    - phase 1, per sample b: h[128,512] = xT[:, 98b:98b+128].T @ Wall
      (2 accumulating fp16 matmuls, f32 PSUM), 2 samples per PSUM tile
    - phase 2, per 8-sample group, per head: 2 accumulating matmuls with
      host-built graph matrices (diag-embed & A_off^T, zero-padded to
      K=128); bias added during the PSUM->SBUF copy on DVE
    - DMA out f32
"""

import os
import sys

import numpy as np

try:
    import concourse.bass as bass  # noqa: F401
except Exception:  # pragma: no cover - fresh grading dir fallback
    for p in ("/opt/trn_rl_repo", "/root/.axon_site/_ro/trn_rl_repo"):
        if os.path.isdir(p) and p not in sys.path:
            sys.path.insert(0, p)
    import concourse.bass as bass  # noqa: F401

# ---------------------------------------------------------------- constants
NLM = 98          # landmarks (graph nodes)
HEADS = 4
CIN = 256
HC = 64
HD = 512          # h width = 2 (s) * 4 (heads) * 64 (d)
B = 1024
NCORES = 8
NS = B // NCORES  # samples per core = 128
P = 128
G = 8             # samples per output group
NGRP = NS // G    # 16 groups per core
OVL = 30          # overlap cols so every phase-1 lhsT can be m=128
NEG = -9e15

CHS = 16                    # samples per xT chunk
NCH = NS // CHS             # 8 chunks
CHW = CHS * NLM             # 1568 cols per chunk (+OVL)
NFT = NS * NLM // P         # 98 flat 128-row tiles
NPAIR = NFT // 2            # 49 transpose pairs
DGF = 14                    # flat tiles per input DMA group
NDG = NFT // DGF            # 7 DMA groups

_CACHE = {}


def _build_nc():
    import concourse.mybir as mybir
    import concourse.tile as tile
    from concourse import bacc

    f16 = mybir.dt.float16
    f32 = mybir.dt.float32

    nc = bacc.Bacc(None, target_bir_lowering=False)

    x = nc.dram_tensor("x", [NS * NLM, CIN], f32, kind="ExternalInput")
    wall = nc.dram_tensor("wall", [P, 2, HD], f16, kind="ExternalInput")
    gmat = nc.dram_tensor("gmat", [P, 2 * HEADS * P], f16, kind="ExternalInput")
    biast = nc.dram_tensor("biast", [NLM, G * 256], f32, kind="ExternalInput")
    ident = nc.dram_tensor("ident", [P, P], f16, kind="ExternalInput")
    out = nc.dram_tensor("out", [NS * NLM, CIN], f32, kind="ExternalOutput")

    with tile.TileContext(nc) as tc:
        with (
            tc.tile_pool(name="const", bufs=1) as constp,
            tc.tile_pool(name="xin", bufs=4) as xinp,
            tc.tile_pool(name="xt", bufs=1) as xtp,
            tc.tile_pool(name="hgrp", bufs=2) as hgp,
            tc.tile_pool(name="osb", bufs=2) as osbp,
            tc.tile_pool(name="ptr", bufs=2, space="PSUM") as ptrp,
            tc.tile_pool(name="phs", bufs=2, space="PSUM") as phsp,
            tc.tile_pool(name="pout", bufs=2, space="PSUM") as poutp,
        ):
            ident_sb = constp.tile([P, P], f16, tag="ident")
            nc.sync.dma_start(ident_sb[:], ident[:])
            ident32_sb = constp.tile([P, P], f32, tag="ident32")
            nc.vector.tensor_copy(ident32_sb[:], ident_sb[:])
            wall_sb = constp.tile([P, 2, HD], f16, tag="wall")
            nc.sync.dma_start(wall_sb[:], wall[:])
            gm_sb = constp.tile([P, 2 * HEADS * P], f16, tag="gmat")
            nc.sync.dma_start(gm_sb[:], gmat[:])
            bias_sb = constp.tile([NLM, G * 256], f32, tag="biast")
            nc.sync.dma_start(bias_sb[:], biast[:])

            xt = [
                xtp.tile([P, 2, CHW + OVL], f16, tag=f"xt{k}", name=f"xt{k}")
                for k in range(NCH)
            ]
            nc.vector.memset(xt[NCH - 1][:, :, CHW:], 0.0)

            bias3 = bias_sb[:].rearrange("p (s c) -> p s c", s=G)

            def route_piece(g0, ptr, off, w):
                """Copy ptr[:, :, off:off+w] (global xT cols [g0,g0+w)) into
                the chunk tiles, including overlap duplication."""
                while w > 0:
                    k = g0 // CHW
                    lo = g0 - k * CHW
                    pw = min(w, CHW - lo)
                    nc.scalar.copy(
                        out=xt[k][:, :, lo : lo + pw],
                        in_=ptr[:, :, off : off + pw],
                    )
                    # overlap region of the previous chunk
                    if k > 0 and lo < OVL:
                        ow = min(pw, OVL - lo)
                        nc.scalar.copy(
                            out=xt[k - 1][:, :, CHW + lo : CHW + lo + ow],
                            in_=ptr[:, :, off : off + ow],
                        )
                    g0 += pw
                    off += pw
                    w -= pw

            def transpose_pair(xin_ap, gft):
                """Transpose 2 flat tiles (xin_ap: [P, 2, CIN] fp16) whose
                first global flat-tile index is gft."""
                ptr = ptrp.tile([P, 2, 2 * P], mybir.dt.float32, tag="ptr")
                ident_ap = (
                    ident32_sb[:]
                    if xin_ap.dtype == mybir.dt.float32
                    else ident_sb[:]
                )
                for a in range(2):
                    for cc in range(2):
                        nc.tensor.matmul(
                            ptr[:, cc, a * P : (a + 1) * P],
                            xin_ap[:, a, cc * P : (cc + 1) * P],
                            ident_ap,
                            start=True,
                            stop=True,
                        )
                route_piece(gft * P, ptr, 0, 2 * P)

            def emit_a_group(dg, split=False):
                """DMA DGF flat x tiles (cast to fp16) and transpose them.
                With split=True use per-pair DMAs so the PE can start as soon
                as the first 256 rows land (cuts kernel-head latency)."""
                base = dg * DGF * P
                if split:
                    # HWDGE f32 loads (no descriptor-gen serialization on the
                    # GpSimd queue) + fp32 transposes; only DGF tiles pay the
                    # 2x fp32 PE rate, and the kernel head shrinks.
                    for pr in range(DGF // 2):
                        xs = xinp.tile([P, 2, CIN], f32, tag="xin0")
                        b0 = base + pr * 2 * P
                        nc.sync.dma_start(
                            xs[:],
                            x[b0 : b0 + 2 * P, :].rearrange(
                                "(t p) c -> p t c", p=P
                            ),
                        )
                        transpose_pair(xs[:], dg * DGF + pr * 2)
                    return
                xin = xinp.tile([P, DGF, CIN], f16, tag="xin")
                nc.gpsimd.dma_start(
                    xin[:],
                    x[base : base + DGF * P, :].rearrange("(t p) c -> p t c", p=P),
                )
                for pr in range(DGF // 2):
                    transpose_pair(
                        xin[:, pr * 2 : pr * 2 + 2, :], dg * DGF + pr * 2
                    )

            def emit_b_group(gi):
                """Phase 1 for G samples, then phase 2 + bias + store."""
                hgrp = hgp.tile([P, G * HD], f16, tag="hgrp")
                ck = (gi * G) // CHS
                for pi in range(G // 2):
                    hps = phsp.tile([P, 2, HD], mybir.dt.float32, tag="hps")
                    for a in range(2):
                        b = gi * G + pi * 2 + a
                        lb = b - ck * CHS
                        for cc in range(2):
                            nc.tensor.matmul(
                                hps[:, a, :],
                                xt[ck][:, cc, NLM * lb : NLM * lb + P],
                                wall_sb[:, cc, :],
                                start=(cc == 0),
                                stop=(cc == 1),
                            )
                    dst = hgrp[:, pi * 2 * HD : (pi + 1) * 2 * HD].rearrange(
                        "p (a f) -> p a f", a=2
                    )
                    if (gi * G // 2 + pi) % 5 < 3:
                        nc.scalar.copy(out=dst, in_=hps[:])
                    else:
                        nc.vector.tensor_copy(dst, hps[:])

                hg3 = hgrp[:].rearrange("p (s f) -> p s f", s=G)
                osb = osbp.tile([NLM, G * 256], mybir.dt.float32, tag="osb")
                osb3 = osb[:].rearrange("p (s c) -> p s c", s=G)
                for hd in range(HEADS):
                    pouts = poutp.tile([P, G * HC], mybir.dt.float32, tag="pout")
                    po3 = pouts[:].rearrange("p (s f) -> p s f", s=G)
                    for prt in range(2):
                        q = hd * 2 + prt
                        nc.tensor.matmul(
                            po3,
                            gm_sb[:, q * P : (q + 1) * P],
                            hg3[:, :, prt * 256 + hd * HC : prt * 256 + (hd + 1) * HC],
                            start=(prt == 0),
                            stop=(prt == 1),
                        )
                    nc.vector.tensor_add(
                        out=osb3[:, :, hd * HC : (hd + 1) * HC],
                        in0=po3[:NLM],
                        in1=bias3[:, :, hd * HC : (hd + 1) * HC],
                    )
                ov = out[gi * G * NLM : (gi + 1) * G * NLM, :].rearrange(
                    "(s i) c -> i s c", s=G
                )
                if gi < NGRP - 1:
                    nc.sync.dma_start(ov, osb3)
                else:
                    # split the final store so the kernel tail is shorter
                    h = G // 2
                    nc.sync.dma_start(ov[:, :h], osb3[:, :h])
                    nc.sync.dma_start(ov[:, h:], osb3[:, h:])

            # chunk k is fully transposed once DMA group ceil((1598+1568k)/1792)
            # has been processed; interleave A and B so PE never starves.
            ready_dg = [
                -(-(CHW * k + CHW + OVL) // (DGF * P)) for k in range(NCH)
            ]  # per chunk, 1-indexed count of A groups needed
            ready_dg[NCH - 1] = NDG
            # Prefetch input DMAs two B-groups ahead: the SWDGE descriptor
            # generation shares the GpSimd FIFO with the bias adds, so a
            # just-in-time DMA would queue behind a ~4.5us add and starve PE.
            emitted = 0
            for gi in range(NGRP):
                need = ready_dg[(min(gi + 2, NGRP - 1) * G) // CHS]
                while emitted < need:
                    emit_a_group(emitted, split=(emitted == 0))
                    emitted += 1
                emit_b_group(gi)

    nc.compile()
    return nc


def _host_prep(W, e, bias, rows, cols):
    """Precompute fp16 device constants from the small parameter tensors."""
    W = np.asarray(W, np.float32)
    e = np.asarray(e, np.float32)
    bias = np.asarray(bias, np.float32)
    rows = np.asarray(rows, np.int64)
    cols = np.asarray(cols, np.int64)

    logits = np.full((HEADS, NLM, NLM), NEG, np.float64)
    logits[:, rows, cols] = e.astype(np.float64)
    m = logits.max(axis=-1, keepdims=True)
    p = np.exp(logits - m)
    A = p / p.sum(axis=-1, keepdims=True)            # [H, N, N]
    dg = np.einsum("hii->hi", A).copy()              # [H, N]
    A_off = A.copy()
    np.einsum("hii->hi", A_off)[:] = 0.0

    # Wall: [c, (s, h, d)] -> chunked [128, 2, 512]
    wr = W.transpose(2, 1, 0, 3).reshape(CIN, 2 * HEADS * HC)   # [c, shd]
    wall = np.ascontiguousarray(
        wr.reshape(2, P, 2 * HEADS * HC).transpose(1, 0, 2)
    ).astype(np.float16)

    # graph matrices, zero-padded to 128 rows & cols: [j, (head, part, i)]
    gm = np.zeros((P, HEADS, 2, P), np.float32)
    idx = np.arange(NLM)
    for h in range(HEADS):
        gm[idx, h, 0, idx] = dg[h]
        gm[:NLM, h, 1, :NLM] = A_off[h].T
    gmat = np.ascontiguousarray(gm.reshape(P, 2 * HEADS * P)).astype(np.float16)

    bcat = bias.reshape(HEADS * HC)                  # col = h*64+d
    biast = np.ascontiguousarray(np.tile(bcat, (NLM, G))).astype(np.float32)

    ident = np.eye(P, dtype=np.float16)
    return {"wall": wall, "gmat": gmat, "biast": biast, "ident": ident}


def kernel(x, W, e, bias, rows, cols):
    from concourse.bass_utils import run_bass_kernel_spmd

    if "nc" not in _CACHE:
        _CACHE["nc"] = _build_nc()
    nc = _CACHE["nc"]

    consts = _host_prep(W, e, bias, rows, cols)
    x = np.ascontiguousarray(np.asarray(x, np.float32)).reshape(B, NLM, CIN)

    in_maps = []
    for ci in range(NCORES):
        shard = np.ascontiguousarray(
            x[ci * NS : (ci + 1) * NS].reshape(NS * NLM, CIN)
        )
        in_maps.append({"x": shard, **consts})

    res = run_bass_kernel_spmd(
        nc,
        in_maps,
        core_ids=list(range(NCORES)),
        trace=bool(int(os.environ.get("KERNEL_TRACE", "0"))),
    )
    _CACHE["last_results"] = res

    out = np.concatenate(
        [r["out"].reshape(NS, NLM, HEADS * HC) for r in res.results], axis=0
    )
    return out



# revision 2
# speedup vs baseline: 1.0134x; 1.0134x over previous
"""MultiHeadSemGConv Trainium2 kernel.

Computes, for x:[B,N,CIN], W:[H,2,CIN,HC], e:[H,N*K], bias:[H,HC],
rows/cols:[N*K] (int32 edge list):

    h = einsum('bnc,hscd->shbnd', x, W)             # two projections per head
    A = softmax(scatter(e at (rows,cols), NEG))     # [H,N,N]
    out[h,b] = diag(A)*h0 + (A - diag)@h1 + bias    # -> [B,N,H*HC]

Strategy: pure data-parallel over batch across 8 NeuronCores.  The tiny
[H,98,98] adjacency softmax is precomputed on host.  Per core:

  - host ships x pre-transposed and pre-cast: xT [256, 12576] fp16
    (halves input HBM traffic; removes all on-device transposes)
  - phase 1, per sample b: h[128,512] = xT[:, 98b:98b+128].T @ Wall
    (2 accumulating fp16 matmuls, f32 PSUM), 2 samples per PSUM tile
  - PSUM -> SBUF fp16 copies split across Scalar/Vector engines into a
    per-group [128, 8, 512] tile whose row 98 holds the bias (loaded
    once; an all-ones row 98 in the graph matrices adds it in phase 2)
  - phase 2, per 8-sample group, per head: 2 accumulating matmuls with
    host-built graph matrices (diag-embed & A_off^T zero-padded to 128,
    plus the bias ones-row)
  - PSUM -> SBUF fp16, DMA out fp16; host upcasts to f32
"""

import os
import sys

import numpy as np

try:
    import concourse.bass as bass  # noqa: F401
except Exception:  # pragma: no cover - fresh grading dir fallback
    for p in ("/opt/trn_rl_repo", "/root/.axon_site/_ro/trn_rl_repo"):
        if os.path.isdir(p) and p not in sys.path:
            sys.path.insert(0, p)
    import concourse.bass as bass  # noqa: F401

# ---------------------------------------------------------------- constants
NLM = 98          # landmarks (graph nodes)
HEADS = 4
CIN = 256
HC = 64
HD = 512          # h width = 2 (s) * 4 (heads) * 64 (d)
B = 1024
NCORES = 8
NS = B // NCORES  # samples per core = 128
P = 128
G = 8             # samples per output group
NGRP = NS // G    # 16 groups per core
GW = G * NLM      # xT cols per group = 784
WPAD = NS * NLM + 32   # 12576: xT padded so the last 128-wide window fits
NEG = -9e15

_CACHE = {}


def _build_nc():
    import concourse.mybir as mybir
    import concourse.tile as tile
    from concourse import bacc

    f16 = mybir.dt.float16
    f32 = mybir.dt.float32

    nc = bacc.Bacc(None, target_bir_lowering=False)

    xt_d = nc.dram_tensor("xt", [2 * P, WPAD], f16, kind="ExternalInput")
    wall = nc.dram_tensor("wall", [P, 2, HD], f16, kind="ExternalInput")
    gmat = nc.dram_tensor("gm", [P, HEADS, 2, P], f16, kind="ExternalInput")
    brow = nc.dram_tensor("brow", [30, G * HD], f16, kind="ExternalInput")
    out = nc.dram_tensor("out", [NS * NLM, CIN], f16, kind="ExternalOutput")

    xt_src = xt_d[:].rearrange("(c p) w -> p c w", p=P)

    with tile.TileContext(nc) as tc:
        with (
            tc.tile_pool(name="const", bufs=1) as constp,
            tc.tile_pool(name="xt", bufs=1) as xtp,
            tc.tile_pool(name="hgrp", bufs=1) as hgp,
            tc.tile_pool(name="osb", bufs=2) as osbp,
            tc.tile_pool(name="phs", bufs=2, space="PSUM") as phsp,
            tc.tile_pool(name="po3", bufs=1, space="PSUM") as po3p,
        ):
            wall_sb = constp.tile([P, 2, HD], f16, tag="wall")
            nc.sync.dma_start(wall_sb[:], wall[:])
            gm_sb = constp.tile([P, HEADS, 2, P], f16, tag="gm")
            nc.sync.dma_start(gm_sb[:], gmat[:])

            # two group buffers; rows 98..127 initialized once (row 98 =
            # bias pattern used by the ones-row in gm, 99..127 zeros)
            hgrp = [
                hgp.tile([P, G, HD], f16, tag=f"hgrp{k}", name=f"hgrp{k}")
                for k in range(2)
            ]
            for k in range(2):
                nc.sync.dma_start(
                    hgrp[k][NLM:P],
                    brow[:].rearrange("r (s f) -> r s f", s=G),
                )

            xt_sb = xtp.tile([P, 2, WPAD], f16, tag="xt")

            def emit_in(g):
                c0 = g * GW
                c1 = WPAD if g == NGRP - 1 else (g + 1) * GW
                nc.sync.dma_start(
                    xt_sb[:, :, c0:c1], xt_src[:, :, c0:c1]
                )

            emit_in(0)
            emit_in(1)

            for gi in range(NGRP):
                if gi + 2 < NGRP:
                    emit_in(gi + 2)
                hg = hgrp[gi % 2]
                # -------- phase 1: project 8 samples, 2 per PSUM tile
                for pi in range(G // 2):
                    hps = phsp.tile([P, 2, HD], f32, tag="hps")
                    for a in range(2):
                        b = gi * G + pi * 2 + a
                        for cc in range(2):
                            nc.tensor.matmul(
                                hps[:, a, :],
                                xt_sb[:, cc, NLM * b : NLM * b + P],
                                wall_sb[:, cc, :],
                                start=(cc == 0),
                                stop=(cc == 1),
                            )
                    nc.scalar.copy(
                        out=hg[0:NLM, 2 * pi, :], in_=hps[0:NLM, 0, :]
                    )
                    nc.vector.tensor_copy(
                        hg[0:NLM, 2 * pi + 1, :], hps[0:NLM, 1, :]
                    )
                # -------- phase 2: graph mix per head (diag & off-diag,
                # bias via the gm ones-row against hgrp row 98)
                po3 = po3p.tile([P, HEADS, HD], f32, tag="po3")
                for hd in range(HEADS):
                    nc.tensor.matmul(
                        po3[:, hd, :],
                        gm_sb[:, hd, 0, :],
                        hg[:, :, hd * HC : (hd + 1) * HC],
                        start=True,
                        stop=False,
                    )
                    nc.tensor.matmul(
                        po3[:, hd, :],
                        gm_sb[:, hd, 1, :],
                        hg[:, :, 256 + hd * HC : 256 + (hd + 1) * HC],
                        start=False,
                        stop=True,
                    )
                osb = osbp.tile([NLM, G, CIN], f16, tag="osb")
                ov_h = osb[:].rearrange("i s (h d) -> i h s d", h=HEADS)
                po_h = po3[:].rearrange("i h (s d) -> i h s d", s=G)
                nc.scalar.copy(out=ov_h[:, 0:2], in_=po_h[0:NLM, 0:2])
                nc.vector.tensor_copy(ov_h[:, 2:4], po_h[0:NLM, 2:4])
                nc.gpsimd.dma_start(
                    out[gi * GW : (gi + 1) * GW, :].rearrange(
                        "(s i) c -> i s c", s=G
                    ),
                    osb[:],
                )

    nc.compile()
    return nc


def _host_prep(W, e, bias, rows, cols):
    """Precompute fp16 device constants from the small parameter tensors."""
    W = np.asarray(W, np.float32)
    e = np.asarray(e, np.float32)
    bias = np.asarray(bias, np.float32)
    rows = np.asarray(rows, np.int64)
    cols = np.asarray(cols, np.int64)

    logits = np.full((HEADS, NLM, NLM), NEG, np.float64)
    logits[:, rows, cols] = e.astype(np.float64)
    m = logits.max(axis=-1, keepdims=True)
    p = np.exp(logits - m)
    A = p / p.sum(axis=-1, keepdims=True)            # [H, N, N]
    dg = np.einsum("hii->hi", A).copy()              # [H, N]
    A_off = A.copy()
    np.einsum("hii->hi", A_off)[:] = 0.0

    # Wall: [c, (s, h, d)] -> chunked [128, 2, 512]
    wr = W.transpose(2, 1, 0, 3).reshape(CIN, 2 * HEADS * HC)   # [c, shd]
    wall = np.ascontiguousarray(
        wr.reshape(2, P, 2 * HEADS * HC).transpose(1, 0, 2)
    ).astype(np.float16)

    # graph matrices [j, head, part, i]: part 0 = diag-embed, part 1 =
    # A_off^T with an all-ones row 98 (adds the bias staged at hgrp[98])
    gm = np.zeros((P, HEADS, 2, P), np.float32)
    idx = np.arange(NLM)
    for h in range(HEADS):
        gm[idx, h, 0, idx] = dg[h]
        gm[:NLM, h, 1, :NLM] = A_off[h].T
        gm[NLM, h, 1, :NLM] = 1.0
    gm = np.ascontiguousarray(gm).astype(np.float16)

    # hgrp rows 98..127: row 98 carries bias at the h1 column block
    brow = np.zeros((30, G, HD), np.float32)
    brow[0, :, 256:512] = bias.reshape(HEADS * HC)
    brow = np.ascontiguousarray(brow.reshape(30, G * HD)).astype(np.float16)

    return {"wall": wall, "gm": gm, "brow": brow}


def kernel(x, W, e, bias, rows, cols):
    from concourse.bass_utils import run_bass_kernel_spmd

    if "nc" not in _CACHE:
        _CACHE["nc"] = _build_nc()
    nc = _CACHE["nc"]

    consts = _host_prep(W, e, bias, rows, cols)
    x = np.ascontiguousarray(np.asarray(x, np.float32)).reshape(
        B * NLM, CIN
    )

    in_maps = []
    for ci in range(NCORES):
        sh = x[ci * NS * NLM : (ci + 1) * NS * NLM]          # [12544, 256]
        xT = np.zeros((CIN, WPAD), np.float16)
        xT[:, : NS * NLM] = sh.T
        in_maps.append({"xt": xT, **consts})

    res = run_bass_kernel_spmd(
        nc,
        in_maps,
        core_ids=list(range(NCORES)),
        trace=bool(int(os.environ.get("KERNEL_TRACE", "0"))),
    )
    _CACHE["last_results"] = res

    out = np.concatenate(
        [
            r["out"].astype(np.float32).reshape(NS, NLM, HEADS * HC)
            for r in res.results
        ],
        axis=0,
    )
    return out


# revision 5
# speedup vs baseline: 1.1551x; 1.1398x over previous
"""MultiHeadSemGConv Trainium2 kernel.

Computes, for x:[B,N,CIN], W:[H,2,CIN,HC], e:[H,N*K], bias:[H,HC],
rows/cols:[N*K] (int32 edge list):

    h = einsum('bnc,hscd->shbnd', x, W)             # two projections per head
    A = softmax(scatter(e at (rows,cols), NEG))     # [H,N,N]
    out[h,b] = diag(A)*h0 + (A - diag)@h1 + bias    # -> [B,N,H*HC]

Strategy: pure data-parallel over batch across 8 NeuronCores.  The tiny
[H,98,98] adjacency softmax is precomputed on host.  Per core:

  - host ships x pre-transposed and pre-cast: xT [256, 12576] fp16
    (halves input HBM traffic; removes all on-device transposes)
  - phase 1, per sample b: h[128,512] = xT[:, 98b:98b+128].T @ Wall
    (2 accumulating fp16 matmuls, f32 PSUM), 2 samples per PSUM tile
  - PSUM -> SBUF fp16 copies split across Scalar/Vector engines into a
    per-group [128, 8, 512] tile whose row 98 holds the bias (loaded
    once; an all-ones row 98 in the graph matrices adds it in phase 2)
  - phase 2, per 8-sample group, per head: 2 accumulating matmuls with
    host-built graph matrices (diag-embed & A_off^T zero-padded to 128,
    plus the bias ones-row)
  - PSUM -> SBUF fp16, DMA out fp16; host upcasts to f32
"""

import os
import sys

import numpy as np

try:
    import concourse.bass as bass  # noqa: F401
except Exception:  # pragma: no cover - fresh grading dir fallback
    for p in ("/opt/trn_rl_repo", "/root/.axon_site/_ro/trn_rl_repo"):
        if os.path.isdir(p) and p not in sys.path:
            sys.path.insert(0, p)
    import concourse.bass as bass  # noqa: F401

# ---------------------------------------------------------------- constants
NLM = 98          # landmarks (graph nodes)
HEADS = 4
CIN = 256
HC = 64
HD = 512          # h width = 2 (s) * 4 (heads) * 64 (d)
B = 1024
NCORES = 8
NS = B // NCORES  # samples per core = 128
P = 128
G = 8             # samples per output group
NGRP = NS // G    # 16 groups per core
GW = G * NLM      # xT cols per group = 784
WPAD = NS * NLM + 32   # 12576: xT padded so the last 128-wide window fits
NEG = -9e15

_CACHE = {}


def _build_nc():
    import concourse.mybir as mybir
    import concourse.tile as tile
    from concourse import bacc

    f16 = mybir.dt.float16
    f32 = mybir.dt.float32

    nc = bacc.Bacc(None, target_bir_lowering=False)

    xt_d = nc.dram_tensor("xt", [2 * P, WPAD], f16, kind="ExternalInput")
    wall = nc.dram_tensor("wall", [P, 2, HD], f16, kind="ExternalInput")
    gmat = nc.dram_tensor("gm", [P, HEADS, 2, P], f16, kind="ExternalInput")
    brow = nc.dram_tensor("brow", [30, G * HD], f16, kind="ExternalInput")
    out = nc.dram_tensor("out", [NS * NLM, CIN], f16, kind="ExternalOutput")

    xt_src = xt_d[:].rearrange("(c p) w -> p c w", p=P)

    with tile.TileContext(nc) as tc:
        with (
            tc.tile_pool(name="const", bufs=1) as constp,
            tc.tile_pool(name="xt", bufs=1) as xtp,
            tc.tile_pool(name="hgrp", bufs=1) as hgp,
            tc.tile_pool(name="osb", bufs=2) as osbp,
            tc.tile_pool(name="phs", bufs=4, space="PSUM") as phsp,
            tc.tile_pool(name="po3", bufs=1, space="PSUM") as po3p,
        ):
            xt_sb = xtp.tile([P, 2, WPAD], f16, tag="xt")

            def emit_in(g, c0=None, c1=None):
                c0 = g * GW if c0 is None else c0
                if c1 is None:
                    c1 = WPAD if g == NGRP - 1 else (g + 1) * GW
                nc.sync.dma_start(
                    xt_sb[:, :, c0:c1], xt_src[:, :, c0:c1]
                )

            # first two samples land first so the PE can start early
            emit_in(0, 0, 2 * NLM)
            wall_sb = constp.tile([P, 2, HD], f16, tag="wall")
            nc.sync.dma_start(wall_sb[:], wall[:])
            emit_in(0, 2 * NLM, GW)
            gm_sb = constp.tile([P, HEADS, 2, P], f16, tag="gm")
            nc.sync.dma_start(gm_sb[:], gmat[:])

            # group buffers; rows 98..127 initialized once (row 98 =
            # bias pattern used by the ones-row in gm, 99..127 zeros)
            NHG = 4
            hgrp = [
                hgp.tile([P, G, HD], f16, tag=f"hgrp{k}", name=f"hgrp{k}")
                for k in range(NHG)
            ]
            emit_in(1)
            for k in range(NHG):
                nc.sync.dma_start(
                    hgrp[k][NLM:P],
                    brow[:].rearrange("r (s f) -> r s f", s=G),
                )

            for gi in range(NGRP):
                if gi + 2 < NGRP:
                    emit_in(gi + 2)
                hg = hgrp[gi % NHG]
                # -------- phase 1: project 8 samples, 1 per PSUM tile
                for si in range(G):
                    b = gi * G + si
                    hps = phsp.tile([P, HD], f32, tag="hps")
                    for cc in range(2):
                        nc.tensor.matmul(
                            hps[:],
                            xt_sb[:, cc, NLM * b : NLM * b + P],
                            wall_sb[:, cc, :],
                            start=(cc == 0),
                            stop=(cc == 1),
                        )
                    if si % 2 == 0:
                        nc.scalar.copy(
                            out=hg[0:NLM, si, :], in_=hps[0:NLM]
                        )
                    else:
                        nc.vector.tensor_copy(
                            hg[0:NLM, si, :], hps[0:NLM]
                        )
                # -------- phase 2: graph mix per head (diag & off-diag,
                # bias via the gm ones-row against hgrp row 98)
                po3 = po3p.tile([P, HEADS, HD], f32, tag="po3")
                for hd in range(HEADS):
                    nc.tensor.matmul(
                        po3[:, hd, :],
                        gm_sb[:, hd, 0, :],
                        hg[:, :, hd * HC : (hd + 1) * HC],
                        start=True,
                        stop=False,
                    )
                    nc.tensor.matmul(
                        po3[:, hd, :],
                        gm_sb[:, hd, 1, :],
                        hg[:, :, 256 + hd * HC : 256 + (hd + 1) * HC],
                        start=False,
                        stop=True,
                    )
                osb = osbp.tile([NLM, G, CIN], f16, tag="osb")
                ov_h = osb[:].rearrange("i s (h d) -> i h s d", h=HEADS)
                po_h = po3[:].rearrange("i h (s d) -> i h s d", s=G)
                nc.scalar.copy(out=ov_h[:, 0:2], in_=po_h[0:NLM, 0:2])
                nc.vector.tensor_copy(ov_h[:, 2:4], po_h[0:NLM, 2:4])
                nc.sync.dma_start(
                    out[gi * GW : (gi + 1) * GW, :].rearrange(
                        "(s i) c -> i s c", s=G
                    ),
                    osb[:],
                )

    nc.compile()
    return nc


def _host_prep(W, e, bias, rows, cols):
    """Precompute fp16 device constants from the small parameter tensors."""
    W = np.asarray(W, np.float32)
    e = np.asarray(e, np.float32)
    bias = np.asarray(bias, np.float32)
    rows = np.asarray(rows, np.int64)
    cols = np.asarray(cols, np.int64)

    logits = np.full((HEADS, NLM, NLM), NEG, np.float64)
    logits[:, rows, cols] = e.astype(np.float64)
    m = logits.max(axis=-1, keepdims=True)
    p = np.exp(logits - m)
    A = p / p.sum(axis=-1, keepdims=True)            # [H, N, N]
    dg = np.einsum("hii->hi", A).copy()              # [H, N]
    A_off = A.copy()
    np.einsum("hii->hi", A_off)[:] = 0.0

    # Wall: [c, (s, h, d)] -> chunked [128, 2, 512]
    wr = W.transpose(2, 1, 0, 3).reshape(CIN, 2 * HEADS * HC)   # [c, shd]
    wall = np.ascontiguousarray(
        wr.reshape(2, P, 2 * HEADS * HC).transpose(1, 0, 2)
    ).astype(np.float16)

    # graph matrices [j, head, part, i]: part 0 = diag-embed, part 1 =
    # A_off^T with an all-ones row 98 (adds the bias staged at hgrp[98])
    gm = np.zeros((P, HEADS, 2, P), np.float32)
    idx = np.arange(NLM)
    for h in range(HEADS):
        gm[idx, h, 0, idx] = dg[h]
        gm[:NLM, h, 1, :NLM] = A_off[h].T
        gm[NLM, h, 1, :NLM] = 1.0
    gm = np.ascontiguousarray(gm).astype(np.float16)

    # hgrp rows 98..127: row 98 carries bias at the h1 column block
    brow = np.zeros((30, G, HD), np.float32)
    brow[0, :, 256:512] = bias.reshape(HEADS * HC)
    brow = np.ascontiguousarray(brow.reshape(30, G * HD)).astype(np.float16)

    return {"wall": wall, "gm": gm, "brow": brow}


def kernel(x, W, e, bias, rows, cols):
    from concourse.bass_utils import run_bass_kernel_spmd

    if "nc" not in _CACHE:
        _CACHE["nc"] = _build_nc()
    nc = _CACHE["nc"]

    consts = _host_prep(W, e, bias, rows, cols)
    x = np.ascontiguousarray(np.asarray(x, np.float32)).reshape(
        B * NLM, CIN
    )

    in_maps = []
    for ci in range(NCORES):
        sh = x[ci * NS * NLM : (ci + 1) * NS * NLM]          # [12544, 256]
        xT = np.zeros((CIN, WPAD), np.float16)
        xT[:, : NS * NLM] = sh.T
        in_maps.append({"xt": xT, **consts})

    res = run_bass_kernel_spmd(
        nc,
        in_maps,
        core_ids=list(range(NCORES)),
        trace=bool(int(os.environ.get("KERNEL_TRACE", "0"))),
    )
    _CACHE["last_results"] = res

    out = np.concatenate(
        [
            r["out"].astype(np.float32).reshape(NS, NLM, HEADS * HC)
            for r in res.results
        ],
        axis=0,
    )
    return out


# revision 9
# speedup vs baseline: 1.2699x; 1.0994x over previous
"""MultiHeadSemGConv Trainium2 kernel.

Computes, for x:[B,N,CIN], W:[H,2,CIN,HC], e:[H,N*K], bias:[H,HC],
rows/cols:[N*K] (int32 edge list):

    h = einsum('bnc,hscd->shbnd', x, W)             # two projections per head
    A = softmax(scatter(e at (rows,cols), NEG))     # [H,N,N]
    out[h,b] = diag(A)*h0 + (A - diag)@h1 + bias    # -> [B,N,H*HC]

Strategy: pure data-parallel over batch across 8 NeuronCores.  The tiny
[H,98,98] adjacency softmax is precomputed on host.  Per core:

  - host ships x pre-transposed and pre-cast: xT [256, 12576] fp16
    (halves input HBM traffic; removes all on-device transposes)
  - phase 1, per sample b: h[128,512] = xT[:, 98b:98b+128].T @ Wall
    (2 accumulating fp16 matmuls, f32 PSUM), 2 samples per PSUM tile
  - PSUM -> SBUF fp16 copies split across Scalar/Vector engines into a
    per-group [128, 8, 512] tile whose row 98 holds the bias (loaded
    once; an all-ones row 98 in the graph matrices adds it in phase 2)
  - phase 2, per 8-sample group, per head: 2 accumulating matmuls with
    host-built graph matrices (diag-embed & A_off^T zero-padded to 128,
    plus the bias ones-row)
  - PSUM -> SBUF fp16, DMA out fp16; host upcasts to f32
"""

import os
import sys

import numpy as np

try:
    import concourse.bass as bass  # noqa: F401
except Exception:  # pragma: no cover - fresh grading dir fallback
    for p in ("/opt/trn_rl_repo", "/root/.axon_site/_ro/trn_rl_repo"):
        if os.path.isdir(p) and p not in sys.path:
            sys.path.insert(0, p)
    import concourse.bass as bass  # noqa: F401

# ---------------------------------------------------------------- constants
NLM = 98          # landmarks (graph nodes)
HEADS = 4
CIN = 256
HC = 64
HD = 512          # h width = 2 (s) * 4 (heads) * 64 (d)
B = 1024
NCORES = 8
NS = B // NCORES  # samples per core = 128
P = 128
G = 8             # samples per output group
NGRP = NS // G    # 16 groups per core
GW = G * NLM      # xT cols per group = 784
WPAD = NS * NLM + 32   # 12576: xT padded so the last 128-wide window fits
NEG = -9e15

_CACHE = {}


def _build_nc():
    import concourse.mybir as mybir
    import concourse.tile as tile
    from concourse import bacc

    f16 = mybir.dt.float16
    f32 = mybir.dt.float32

    nc = bacc.Bacc(None, target_bir_lowering=False)

    xt_d = nc.dram_tensor("xt", [2 * P, WPAD], f16, kind="ExternalInput")
    wall = nc.dram_tensor("wall", [P, 2, HD], f16, kind="ExternalInput")
    gmat = nc.dram_tensor("gm", [P, HEADS, P], f16, kind="ExternalInput")
    dgv_d = nc.dram_tensor("dgv", [NLM, HEADS], f16, kind="ExternalInput")
    brow = nc.dram_tensor("brow", [30, G * HD], f16, kind="ExternalInput")
    out = nc.dram_tensor("out", [NS * NLM, CIN], f16, kind="ExternalOutput")

    xt_src = xt_d[:].rearrange("(c p) w -> p c w", p=P)

    with tile.TileContext(nc) as tc:
        with (
            tc.tile_pool(name="const", bufs=1) as constp,
            tc.tile_pool(name="xt", bufs=1) as xtp,
            tc.tile_pool(name="hgrp", bufs=1) as hgp,
            tc.tile_pool(name="osb", bufs=2) as osbp,
            tc.tile_pool(name="phs", bufs=4, space="PSUM") as phsp,
            tc.tile_pool(name="po3", bufs=1, space="PSUM") as po3p,
        ):
            xt_sb = xtp.tile([P, 2, WPAD], f16, tag="xt")

            def emit_in(g, c0=None, c1=None):
                c0 = g * GW if c0 is None else c0
                if c1 is None:
                    c1 = WPAD if g == NGRP - 1 else (g + 1) * GW
                nc.sync.dma_start(
                    xt_sb[:, :, c0:c1], xt_src[:, :, c0:c1]
                )

            # first two samples land first so the PE can start early
            emit_in(0, 0, 2 * NLM)
            wall_sb = constp.tile([P, 2, HD], f16, tag="wall")
            nc.sync.dma_start(wall_sb[:], wall[:])
            emit_in(0, 2 * NLM, GW)
            gm_sb = constp.tile([P, HEADS, P], f16, tag="gm")
            nc.sync.dma_start(gm_sb[:], gmat[:])
            dg_sb = constp.tile([NLM, HEADS], f16, tag="dgv")
            nc.sync.dma_start(dg_sb[:], dgv_d[:])

            # group buffers; rows 98..127 initialized once (row 98 =
            # bias pattern used by the ones-row in gm, 99..127 zeros)
            NHG = 4
            hgrp = [
                hgp.tile([P, G, HD], f16, tag=f"hgrp{k}", name=f"hgrp{k}")
                for k in range(NHG)
            ]
            emit_in(1)
            for k in range(NHG):
                nc.sync.dma_start(
                    hgrp[k][NLM:P],
                    brow[:].rearrange("r (s f) -> r s f", s=G),
                )

            for gi in range(NGRP):
                if gi + 2 < NGRP:
                    emit_in(gi + 2)
                hg = hgrp[gi % NHG]
                # -------- phase 1: project 8 samples, 1 per PSUM tile
                for si in range(G):
                    b = gi * G + si
                    hps = phsp.tile([P, HD], f32, tag="hps")
                    for cc in range(2):
                        nc.tensor.matmul(
                            hps[:],
                            xt_sb[:, cc, NLM * b : NLM * b + P],
                            wall_sb[:, cc, :],
                            start=(cc == 0),
                            stop=(cc == 1),
                        )
                    if si < 2:
                        nc.vector.tensor_copy(
                            hg[0:NLM, si, :], hps[0:NLM]
                        )
                    else:
                        nc.scalar.copy(
                            out=hg[0:NLM, si, :], in_=hps[0:NLM]
                        )
                # -------- phase 2: off-diag graph mix per head (bias via
                # the gm ones-row against hgrp row 98); then merge the
                # diag term on DVE: osb = h0*dg + (A_off@h1 + bias)
                po3 = po3p.tile([P, HEADS, HD], f32, tag="po3")
                osb = osbp.tile([NLM, G, CIN], f16, tag="osb")
                ov_h = osb[:].rearrange("i s (h d) -> i h s d", h=HEADS)
                po_h = po3[:].rearrange("i h (s d) -> i h s d", s=G)
                # split the final group so its tail chain is shorter
                nhalf = 2 if gi == NGRP - 1 else 1
                for half in range(nhalf):
                    s0 = 0 if nhalf == 1 else half * (G // 2)
                    s1 = G if nhalf == 1 else (half + 1) * (G // 2)
                    for hd in range(HEADS):
                        nc.tensor.matmul(
                            po_h[:, hd, s0:s1],
                            gm_sb[:, hd, :],
                            hg[:, s0:s1, 256 + hd * HC : 256 + (hd + 1) * HC],
                            start=True,
                            stop=True,
                        )
                    for hd in range(HEADS):
                        nc.vector.scalar_tensor_tensor(
                            out=ov_h[:, hd, s0:s1],
                            in0=hg[0:NLM, s0:s1, hd * HC : (hd + 1) * HC],
                            scalar=dg_sb[:, hd : hd + 1],
                            in1=po_h[0:NLM, hd, s0:s1],
                            op0=mybir.AluOpType.mult,
                            op1=mybir.AluOpType.add,
                        )
                    nc.sync.dma_start(
                        out[
                            gi * GW + s0 * NLM : gi * GW + s1 * NLM, :
                        ].rearrange("(s i) c -> i s c", s=s1 - s0),
                        osb[:, s0:s1],
                    )

    nc.compile()
    return nc


def _host_prep(W, e, bias, rows, cols):
    """Precompute fp16 device constants from the small parameter tensors."""
    W = np.asarray(W, np.float32)
    e = np.asarray(e, np.float32)
    bias = np.asarray(bias, np.float32)
    rows = np.asarray(rows, np.int64)
    cols = np.asarray(cols, np.int64)

    logits = np.full((HEADS, NLM, NLM), NEG, np.float64)
    logits[:, rows, cols] = e.astype(np.float64)
    m = logits.max(axis=-1, keepdims=True)
    p = np.exp(logits - m)
    A = p / p.sum(axis=-1, keepdims=True)            # [H, N, N]
    dg = np.einsum("hii->hi", A).copy()              # [H, N]
    A_off = A.copy()
    np.einsum("hii->hi", A_off)[:] = 0.0

    # Wall: [c, (s, h, d)] -> chunked [128, 2, 512]
    wr = W.transpose(2, 1, 0, 3).reshape(CIN, 2 * HEADS * HC)   # [c, shd]
    wall = np.ascontiguousarray(
        wr.reshape(2, P, 2 * HEADS * HC).transpose(1, 0, 2)
    ).astype(np.float16)

    # graph matrices [j, head, i]: A_off^T with an all-ones row 98
    # (adds the bias staged at hgrp[98]); the diag term is applied on
    # DVE via the dgv per-partition scale vector
    gm = np.zeros((P, HEADS, P), np.float32)
    for h in range(HEADS):
        gm[:NLM, h, :NLM] = A_off[h].T
        gm[NLM, h, :NLM] = 1.0
    gm = np.ascontiguousarray(gm).astype(np.float16)

    dgv = np.ascontiguousarray(dg.T).astype(np.float16)     # [98, H]

    # hgrp rows 98..127: row 98 carries bias at the h1 column block
    brow = np.zeros((30, G, HD), np.float32)
    brow[0, :, 256:512] = bias.reshape(HEADS * HC)
    brow = np.ascontiguousarray(brow.reshape(30, G * HD)).astype(np.float16)

    return {"wall": wall, "gm": gm, "dgv": dgv, "brow": brow}


def kernel(x, W, e, bias, rows, cols):
    from concourse.bass_utils import run_bass_kernel_spmd

    if "nc" not in _CACHE:
        _CACHE["nc"] = _build_nc()
    nc = _CACHE["nc"]

    consts = _host_prep(W, e, bias, rows, cols)
    x = np.ascontiguousarray(np.asarray(x, np.float32)).reshape(
        B * NLM, CIN
    )

    in_maps = []
    for ci in range(NCORES):
        sh = x[ci * NS * NLM : (ci + 1) * NS * NLM]          # [12544, 256]
        xT = np.zeros((CIN, WPAD), np.float16)
        xT[:, : NS * NLM] = sh.T
        in_maps.append({"xt": xT, **consts})

    res = run_bass_kernel_spmd(
        nc,
        in_maps,
        core_ids=list(range(NCORES)),
        trace=bool(int(os.environ.get("KERNEL_TRACE", "0"))),
    )
    _CACHE["last_results"] = res

    out = np.concatenate(
        [
            r["out"].astype(np.float32).reshape(NS, NLM, HEADS * HC)
            for r in res.results
        ],
        axis=0,
    )
    return out


# revision 15
# speedup vs baseline: 1.4004x; 1.1028x over previous
"""MultiHeadSemGConv Trainium2 kernel.

Computes, for x:[B,N,CIN], W:[H,2,CIN,HC], e:[H,N*K], bias:[H,HC],
rows/cols:[N*K] (int32 edge list):

    h = einsum('bnc,hscd->shbnd', x, W)             # two projections per head
    A = softmax(scatter(e at (rows,cols), NEG))     # [H,N,N]
    out[h,b] = diag(A)*h0 + (A - diag)@h1 + bias    # -> [B,N,H*HC]

Strategy: pure data-parallel over batch across 8 NeuronCores.  The tiny
[H,98,98] adjacency softmax is precomputed on host.  Per core:

  - host ships x pre-transposed and pre-cast: xT [256, 12576] fp16
    (halves input HBM traffic; removes all on-device transposes)
  - phase 1, per sample b: h[128,512] = xT[:, 98b:98b+128].T @ Wall
    (2 accumulating fp16 matmuls, f32 PSUM), 2 samples per PSUM tile
  - PSUM -> SBUF fp16 copies split across Scalar/Vector engines into a
    per-group [128, 8, 512] tile whose row 98 holds the bias (loaded
    once; an all-ones row 98 in the graph matrices adds it in phase 2)
  - phase 2, per 8-sample group, per head: 2 accumulating matmuls with
    host-built graph matrices (diag-embed & A_off^T zero-padded to 128,
    plus the bias ones-row)
  - PSUM -> SBUF fp16, DMA out fp16; host upcasts to f32
"""

import os
import sys

import numpy as np

try:
    import concourse.bass as bass  # noqa: F401
except Exception:  # pragma: no cover - fresh grading dir fallback
    for p in ("/opt/trn_rl_repo", "/root/.axon_site/_ro/trn_rl_repo"):
        if os.path.isdir(p) and p not in sys.path:
            sys.path.insert(0, p)
    import concourse.bass as bass  # noqa: F401

# ---------------------------------------------------------------- constants
NLM = 98          # landmarks (graph nodes)
HEADS = 4
CIN = 256
HC = 64
HD = 512          # h width = 2 (s) * 4 (heads) * 64 (d)
B = 1024
NCORES = 8
NS = B // NCORES  # samples per core = 128
P = 128
G = 8             # samples per output group
NGRP = NS // G    # 16 groups per core
GW = G * NLM      # xT cols per group = 784
WPAD = NS * NLM + 32   # 12576: xT padded so the last 128-wide window fits
NEG = -9e15

_CACHE = {}


def _build_nc():
    import concourse.mybir as mybir
    import concourse.tile as tile
    from concourse import bacc

    f16 = mybir.dt.float16
    f32 = mybir.dt.float32

    nc = bacc.Bacc(None, target_bir_lowering=False)

    xt_d = nc.dram_tensor("xt", [2 * P, WPAD], f16, kind="ExternalInput")
    wall = nc.dram_tensor("wall", [P, 2, HD], f16, kind="ExternalInput")
    gmat = nc.dram_tensor("gm", [P, HEADS, P], f16, kind="ExternalInput")
    gmd_d = nc.dram_tensor("gmd", [P, HEADS, P], f16, kind="ExternalInput")
    dgv_d = nc.dram_tensor("dgv", [NLM, HEADS], f16, kind="ExternalInput")
    brow = nc.dram_tensor("brow", [30, G * HD], f16, kind="ExternalInput")
    out = nc.dram_tensor("out", [NS * NLM, CIN], f16, kind="ExternalOutput")

    xt_src = xt_d[:].rearrange("(c p) w -> p c w", p=P)

    with tile.TileContext(nc) as tc:
        with (
            tc.tile_pool(name="const", bufs=1) as constp,
            tc.tile_pool(name="xt", bufs=1) as xtp,
            tc.tile_pool(name="hgrp", bufs=1) as hgp,
            tc.tile_pool(name="osb", bufs=2) as osbp,
            tc.tile_pool(name="phs", bufs=4, space="PSUM") as phsp,
            tc.tile_pool(name="po3", bufs=1, space="PSUM") as po3p,
        ):
            xt_sb = xtp.tile([P, 2, WPAD], f16, tag="xt")

            def emit_in(c0, c1):
                nc.sync.dma_start(
                    xt_sb[:, :, c0:c1], xt_src[:, :, c0:c1]
                )

            def emit_chunk(c):
                # chunk c covers groups 2c, 2c+1
                emit_in(2 * c * GW, WPAD if c == 7 else 2 * (c + 1) * GW)

            # first two samples land first so the PE can start early
            emit_in(0, 2 * NLM)
            wall_sb = constp.tile([P, 2, HD], f16, tag="wall")
            nc.sync.dma_start(wall_sb[:], wall[:])
            emit_in(2 * NLM, 2 * GW)
            gm_sb = constp.tile([P, HEADS, P], f16, tag="gm")
            nc.sync.dma_start(gm_sb[:], gmat[:])
            dg_sb = constp.tile([NLM, HEADS], f16, tag="dgv")
            nc.sync.dma_start(dg_sb[:], dgv_d[:])
            emit_chunk(1)
            gmd_sb = constp.tile([P, HEADS, P], f16, tag="gmd")
            nc.sync.dma_start(gmd_sb[:], gmd_d[:])

            # group buffers; rows 98..127 initialized once (row 98 =
            # bias pattern used by the ones-row in gm, 99..127 zeros)
            NHG = 4
            hgrp = [
                hgp.tile([P, G, HD], f16, tag=f"hgrp{k}", name=f"hgrp{k}")
                for k in range(NHG)
            ]
            emit_chunk(2)
            for k in range(NHG):
                nc.sync.dma_start(
                    hgrp[k][NLM:P],
                    brow[:].rearrange("r (s f) -> r s f", s=G),
                )

            for gi in range(NGRP):
                if gi % 2 == 0 and gi // 2 + 3 <= 7:
                    emit_chunk(gi // 2 + 3)
                hg = hgrp[gi % NHG]
                # -------- phase 1: project 8 samples, 1 per PSUM tile
                for si in range(G):
                    b = gi * G + si
                    hps = phsp.tile([P, HD], f32, tag="hps")
                    for cc in range(2):
                        nc.tensor.matmul(
                            hps[:],
                            xt_sb[:, cc, NLM * b : NLM * b + P],
                            wall_sb[:, cc, :],
                            start=(cc == 0),
                            stop=(cc == 1),
                        )
                    ndve = 4 if gi == NGRP - 1 else 2
                    if si < ndve:
                        nc.vector.tensor_copy(
                            hg[0:NLM, si, :], hps[0:NLM]
                        )
                    else:
                        nc.scalar.copy(
                            out=hg[0:NLM, si, :], in_=hps[0:NLM]
                        )
                # -------- phase 2: off-diag graph mix per head (bias via
                # the gm ones-row against hgrp row 98); then merge the
                # diag term on DVE: osb = h0*dg + (A_off@h1 + bias)
                po3 = po3p.tile([P, HEADS, HD], f32, tag="po3")
                osb = osbp.tile([NLM, G, CIN], f16, tag="osb")
                ov_h = osb[:].rearrange("i s (h d) -> i h s d", h=HEADS)
                po_h = po3[:].rearrange("i h (s d) -> i h s d", s=G)
                # split the final group so its tail chain is shorter; it
                # also folds the diag term back into matmuls so the merge
                # is a pair of parallel plain copies (no serial STT chain)
                last = gi == NGRP - 1
                nhalf = 2 if last else 1
                for half in range(nhalf):
                    s0 = 0 if nhalf == 1 else half * (G // 2)
                    s1 = G if nhalf == 1 else (half + 1) * (G // 2)
                    for hd in range(HEADS):
                        nc.tensor.matmul(
                            po_h[:, hd, s0:s1],
                            gm_sb[:, hd, :],
                            hg[:, s0:s1, 256 + hd * HC : 256 + (hd + 1) * HC],
                            start=True,
                            stop=not last,
                        )
                        if last:
                            nc.tensor.matmul(
                                po_h[:, hd, s0:s1],
                                gmd_sb[:, hd, :],
                                hg[:, s0:s1, hd * HC : (hd + 1) * HC],
                                start=False,
                                stop=True,
                            )
                    if last:
                        nc.scalar.copy(
                            out=ov_h[:, 0:2, s0:s1], in_=po_h[0:NLM, 0:2, s0:s1]
                        )
                        nc.vector.tensor_copy(
                            ov_h[:, 2:4, s0:s1], po_h[0:NLM, 2:4, s0:s1]
                        )
                    else:
                        for hd in range(HEADS):
                            nc.vector.scalar_tensor_tensor(
                                out=ov_h[:, hd, s0:s1],
                                in0=hg[0:NLM, s0:s1, hd * HC : (hd + 1) * HC],
                                scalar=dg_sb[:, hd : hd + 1],
                                in1=po_h[0:NLM, hd, s0:s1],
                                op0=mybir.AluOpType.mult,
                                op1=mybir.AluOpType.add,
                            )
                    nc.sync.dma_start(
                        out[
                            gi * GW + s0 * NLM : gi * GW + s1 * NLM, :
                        ].rearrange("(s i) c -> i s c", s=s1 - s0),
                        osb[:, s0:s1],
                    )

    nc.compile()
    return nc


def _host_prep(W, e, bias, rows, cols):
    """Precompute fp16 device constants from the small parameter tensors."""
    W = np.asarray(W, np.float32)
    e = np.asarray(e, np.float32)
    bias = np.asarray(bias, np.float32)
    rows = np.asarray(rows, np.int64)
    cols = np.asarray(cols, np.int64)

    logits = np.full((HEADS, NLM, NLM), NEG, np.float64)
    logits[:, rows, cols] = e.astype(np.float64)
    m = logits.max(axis=-1, keepdims=True)
    p = np.exp(logits - m)
    A = p / p.sum(axis=-1, keepdims=True)            # [H, N, N]
    dg = np.einsum("hii->hi", A).copy()              # [H, N]
    A_off = A.copy()
    np.einsum("hii->hi", A_off)[:] = 0.0

    # Wall: [c, (s, h, d)] -> chunked [128, 2, 512]
    wr = W.transpose(2, 1, 0, 3).reshape(CIN, 2 * HEADS * HC)   # [c, shd]
    wall = np.ascontiguousarray(
        wr.reshape(2, P, 2 * HEADS * HC).transpose(1, 0, 2)
    ).astype(np.float16)

    # graph matrices [j, head, i]: A_off^T with an all-ones row 98
    # (adds the bias staged at hgrp[98]); the diag term is applied on
    # DVE via the dgv per-partition scale vector
    gm = np.zeros((P, HEADS, P), np.float32)
    gmd = np.zeros((P, HEADS, P), np.float32)
    idx = np.arange(NLM)
    for h in range(HEADS):
        gm[:NLM, h, :NLM] = A_off[h].T
        gm[NLM, h, :NLM] = 1.0
        gmd[idx, h, idx] = dg[h]
    gm = np.ascontiguousarray(gm).astype(np.float16)
    gmd = np.ascontiguousarray(gmd).astype(np.float16)

    dgv = np.ascontiguousarray(dg.T).astype(np.float16)     # [98, H]

    # hgrp rows 98..127: row 98 carries bias at the h1 column block
    brow = np.zeros((30, G, HD), np.float32)
    brow[0, :, 256:512] = bias.reshape(HEADS * HC)
    brow = np.ascontiguousarray(brow.reshape(30, G * HD)).astype(np.float16)

    return {"wall": wall, "gm": gm, "gmd": gmd, "dgv": dgv, "brow": brow}


def kernel(x, W, e, bias, rows, cols):
    from concourse.bass_utils import run_bass_kernel_spmd

    if "nc" not in _CACHE:
        _CACHE["nc"] = _build_nc()
    nc = _CACHE["nc"]

    consts = _host_prep(W, e, bias, rows, cols)
    x = np.ascontiguousarray(np.asarray(x, np.float32)).reshape(
        B * NLM, CIN
    )

    in_maps = []
    for ci in range(NCORES):
        sh = x[ci * NS * NLM : (ci + 1) * NS * NLM]          # [12544, 256]
        xT = np.zeros((CIN, WPAD), np.float16)
        xT[:, : NS * NLM] = sh.T
        in_maps.append({"xt": xT, **consts})

    res = run_bass_kernel_spmd(
        nc,
        in_maps,
        core_ids=list(range(NCORES)),
        trace=bool(int(os.environ.get("KERNEL_TRACE", "0"))),
    )
    _CACHE["last_results"] = res

    out = np.concatenate(
        [
            r["out"].astype(np.float32).reshape(NS, NLM, HEADS * HC)
            for r in res.results
        ],
        axis=0,
    )
    return out
